# revision 6
# baseline (speedup 1.0000x reference)
"""CrossNet kernel for Trainium2 (8 NeuronCores, pure data parallel over batch).

Math: reference computes, for i in 0..2:
    s_i = x_k @ w_i          (per-row dot)
    x_k = x * s_i + b_i + x_k
and returns the three intermediate x_k.

Flattened (by induction):  x_k = x * S_k + B_k + x, with
    S_{k+1} = S_k + s_k,  B_k = cumsum(b)[k-1],
    s_k = (1 + S_k) * c_k + d_k,  c_k = x @ w_k,  d_k = B_k @ w_k.
Device kernel per 128-row tile: 3 fused multiply+reduce dots (c_j), a tiny
per-row recurrence producing ts_i = 1 + S_{i+1}, and out_i = x*ts_i + cumb_i.

All tensor data is bf16 on device (rel tolerance is 2e-2; bf16 keeps the
norm-wise error ~5e-3 while halving HBM traffic and doubling DVE rates).
Work is spread across engines so it hides under the DMA stream:
  dots    -> DVE tensor_tensor_reduce (1x) and/or GPSIMD fused stt
  out 'P' -> PE diag(t)*x + ones (x) cumb -> PSUM, ScalarE copy to SBUF bf16
  out 'V' -> DVE tensor_scalar (4x) then tensor_tensor add (2x)
  out 'S' -> ScalarE act-mult, GPSIMD tensor_add
  out 'F' -> DVE fused scalar_tensor_tensor (1x)
  recurrence + diag builds -> ScalarE activation
"""

import os

import numpy as np

B, N, ORDER, NCORES = 4096, 4096, 3, 8
ROWS = B // NCORES  # 512 rows per core
P = 128
NT = ROWS // P  # 4 partition-tiles per core

# per-output path: P = PE+ScalarE, V = DVE ts+tt, S = ScalarE+GPSIMD, F = DVE stt
OUTS = os.environ.get("CK_OUTS", "PPP")
HOST_DOTS = os.environ.get("CK_HOST_DOTS", "0") == "1"
XBUFS = int(os.environ.get("CK_XBUFS", "3"))
OBUFS = int(os.environ.get("CK_OBUFS", "4"))
PSBUFS = int(os.environ.get("CK_PSBUFS", "2"))
SCRBUFS = int(os.environ.get("CK_SCRBUFS", "2"))

_prog_cache = {}


def _build_program():
    from contextlib import ExitStack

    import concourse.bacc as bacc
    import concourse.mybir as mybir
    import concourse.tile as tile

    f32 = mybir.dt.float32
    bf16 = mybir.dt.bfloat16
    Alu = mybir.AluOpType

    nc = bacc.Bacc("TRN2")
    xs = nc.dram_tensor("xs", [ROWS, N], bf16, kind="ExternalInput")
    wr = nc.dram_tensor("wr", [ORDER, N], bf16, kind="ExternalInput")
    cb = nc.dram_tensor("cb", [ORDER, N], bf16, kind="ExternalInput")
    dd = nc.dram_tensor("dd", [P, ORDER], f32, kind="ExternalInput")
    eye = nc.dram_tensor("eye", [P, P], bf16, kind="ExternalInput")
    if HOST_DOTS:
        tsd = nc.dram_tensor("tsd", [P, NT * ORDER], f32, kind="ExternalInput")
    out = nc.dram_tensor("out", [ORDER, ROWS, N], bf16, kind="ExternalOutput")

    HALF = 2048  # psum tile free size (4 banks)

    need_cbb = sorted({i for i in range(ORDER) if OUTS[i] in "VSF"})
    need_wb = [] if HOST_DOTS else list(range(ORDER))

    with ExitStack() as ctx:
        tc = ctx.enter_context(tile.TileContext(nc))
        consts = ctx.enter_context(tc.tile_pool(name="consts", bufs=1))
        xpool = ctx.enter_context(tc.tile_pool(name="xpool", bufs=XBUFS))
        small = ctx.enter_context(tc.tile_pool(name="small", bufs=2))
        opool = ctx.enter_context(tc.tile_pool(name="opool", bufs=OBUFS))
        psum = ctx.enter_context(tc.tile_pool(name="psum", bufs=PSBUFS, space="PSUM"))
        scratchpool = ctx.enter_context(tc.tile_pool(name="scratch", bufs=SCRBUFS))

        # w and cumb rows packed at partition bases {0, 32, 64} — the only
        # bases matmul operands may start at. One all-ones tile serves as the
        # broadcast lhsT at any of those bases. Staging rows ride in opool
        # slots: fully consumed by the setup broadcasts before the first ob.
        wpack = opool.tile([2 * 32 + 1, N], bf16, tag="ob")
        cpack = opool.tile([2 * 32 + 1, N], bf16, tag="ob")
        for j in range(ORDER):
            nc.sync.dma_start(out=wpack[32 * j : 32 * j + 1, :], in_=wr[j : j + 1, :])
            nc.sync.dma_start(out=cpack[32 * j : 32 * j + 1, :], in_=cb[j : j + 1, :])
        dd_t = consts.tile([P, ORDER], f32, tag="dd")
        nc.sync.dma_start(out=dd_t, in_=dd[:, :])
        eye_t = consts.tile([P, P], bf16, tag="eye")
        nc.sync.dma_start(out=eye_t, in_=eye[:, :])
        if HOST_DOTS:
            tsd_t = consts.tile([P, NT * ORDER], f32, tag="tsd")
            nc.sync.dma_start(out=tsd_t, in_=tsd[:, :])
        opack = consts.tile([2 * 32 + 1, P], bf16, tag="opack")
        nc.vector.memset(opack, 1.0)

        def row_of(pack, j):
            return pack[32 * j : 32 * j + 1, :]

        def one_row(j):
            return opack[32 * j : 32 * j + 1, :]

        def pe_broadcast(dst, pack, j):
            # dst[128, N] = broadcast of pack row j via ones-matmul.
            for h in range(N // HALF):
                pt = psum.tile([P, HALF], f32, tag="ps")
                for q in range(HALF // 512):
                    sl = slice(h * HALF + q * 512, h * HALF + (q + 1) * 512)
                    nc.tensor.matmul(
                        pt[:, q * 512 : (q + 1) * 512],
                        lhsT=one_row(j),
                        rhs=row_of(pack, j)[:, sl],
                        start=True,
                        stop=True,
                    )
                nc.scalar.copy(dst[:, h * HALF : (h + 1) * HALF], pt)

        wb = {
            j: consts.tile([P, N], bf16, tag=f"wb{j}", name=f"wb{j}")
            for j in need_wb
        }
        cbb = {
            i: consts.tile([P, N], bf16, tag=f"cbb{i}", name=f"cbb{i}")
            for i in need_cbb
        }
        # interleave so the tiles phase A needs first are built first
        build = [(wb[j], wpack, j) for j in need_wb]
        build += [(cbb[i], cpack, i) for i in need_cbb]
        for dst, pack, j in build:
            pe_broadcast(dst, pack, j)

        for k in range(NT):
            rows = slice(k * P, (k + 1) * P)
            x_t = xpool.tile([P, N], bf16, tag="x")
            nc.sync.dma_start(out=x_t, in_=xs[rows, :])

            if HOST_DOTS:
                ts = [tsd_t[:, k * ORDER + i : k * ORDER + i + 1] for i in range(ORDER)]
            else:
                # phase A: c_j = sum_n x * w_j (fused multiply+reduce)
                cs = []
                for j in range(ORDER):
                    cj = small.tile([P, 1], f32, tag=f"c{j}")
                    scr = scratchpool.tile([P, N], bf16, tag="scr")
                    nc.vector.scalar_tensor_tensor(
                        out=scr,
                        in0=x_t,
                        scalar=1.0,
                        in1=wb[j],
                        op0=Alu.mult,
                        op1=Alu.mult,
                        accum_out=cj,
                    )
                    cs.append(cj)

                # recurrence on ScalarE:
                # ts0 = 1 + c0 ; s_i = ts_{i-1}*c_i + d_i ; ts_i = ts_{i-1} + s_i
                ts = []
                t0 = small.tile([P, 1], f32, tag="t0")
                nc.scalar.add(t0, cs[0], 1.0)
                ts.append(t0)
                for i in range(1, ORDER):
                    si = small.tile([P, 1], f32, tag=f"s{i}")
                    nc.scalar.activation(
                        out=si,
                        in_=cs[i],
                        func=mybir.ActivationFunctionType.Identity,
                        bias=dd_t[:, i : i + 1],
                        scale=ts[i - 1],
                    )
                    ti = small.tile([P, 1], f32, tag=f"t{i}")
                    nc.scalar.activation(
                        out=ti,
                        in_=si,
                        func=mybir.ActivationFunctionType.Identity,
                        bias=ts[i - 1],
                        scale=1.0,
                    )
                    ts.append(ti)

            # phase C: out_i = x * ts_i + cumb_i
            for i in range(ORDER):
                ob = opool.tile([P, N], bf16, tag="ob")
                if OUTS[i] == "V":
                    nc.vector.tensor_scalar(
                        out=ob, in0=x_t, scalar1=ts[i], scalar2=None, op0=Alu.mult
                    )
                    nc.vector.tensor_tensor(out=ob, in0=ob, in1=cbb[i], op=Alu.add)
                elif OUTS[i] == "F":
                    nc.vector.scalar_tensor_tensor(
                        out=ob,
                        in0=x_t,
                        scalar=ts[i],
                        in1=cbb[i],
                        op0=Alu.mult,
                        op1=Alu.add,
                    )
                elif OUTS[i] == "S":
                    nc.scalar.mul(ob, x_t, ts[i])
                    nc.gpsimd.tensor_add(ob, ob, cbb[i])
                else:  # 'P'
                    dg = small.tile([P, P], bf16, tag=f"dg{i}")
                    nc.scalar.mul(dg, eye_t, ts[i])
                    for h in range(N // HALF):
                        pt = psum.tile([P, HALF], f32, tag="ps")
                        for q in range(HALF // 512):
                            sl = slice(h * HALF + q * 512, h * HALF + (q + 1) * 512)
                            nc.tensor.matmul(
                                pt[:, q * 512 : (q + 1) * 512],
                                lhsT=one_row(i),
                                rhs=row_of(cpack, i)[:, sl],
                                start=True,
                                stop=False,
                            )
                        for q in range(HALF // 512):
                            sl = slice(h * HALF + q * 512, h * HALF + (q + 1) * 512)
                            nc.tensor.matmul(
                                pt[:, q * 512 : (q + 1) * 512],
                                lhsT=dg,
                                rhs=x_t[:, sl],
                                start=False,
                                stop=True,
                            )
                        nc.scalar.copy(ob[:, h * HALF : (h + 1) * HALF], pt)
                nc.sync.dma_start(out=out[i, rows, :], in_=ob)

    nc.finalize()
    return nc


def _get_program():
    if "nc" not in _prog_cache:
        _prog_cache["nc"] = _build_program()
    return _prog_cache["nc"]


def _prep_inputs(x, w, b):
    import ml_dtypes

    bf16 = ml_dtypes.bfloat16
    x = np.asarray(x, dtype=np.float32)
    w_r = np.asarray(w, dtype=np.float32).reshape(ORDER, N)
    b_r = np.asarray(b, dtype=np.float32).reshape(ORDER, N)
    cumb = np.cumsum(b_r, axis=0).astype(np.float32)  # cumb[i] = b_0 + ... + b_i
    d = np.zeros(ORDER, dtype=np.float64)
    for i in range(1, ORDER):
        d[i] = cumb[i - 1].astype(np.float64) @ w_r[i].astype(np.float64)
    dd = np.tile(d.astype(np.float32)[None, :], (P, 1))
    eye = np.eye(P, dtype=bf16)

    x_bf = np.ascontiguousarray(x.astype(bf16))
    shared = {
        "wr": np.ascontiguousarray(w_r.astype(bf16)),
        "cb": np.ascontiguousarray(cumb.astype(bf16)),
        "dd": np.ascontiguousarray(dd),
        "eye": eye,
    }

    ts_full = None
    if HOST_DOTS:
        # c_j = x_bf16 @ w_bf16 in f32 (mirrors device arithmetic), then the
        # tiny recurrence; ts[i] per row staged per-core as [P, NT*ORDER].
        xb = x_bf.astype(np.float32)
        wbf = np.asarray(shared["wr"], dtype=np.float32)
        c = xb @ wbf.T  # [B, 3]
        ts_full = np.zeros((B, ORDER), dtype=np.float32)
        t = 1.0 + c[:, 0]
        ts_full[:, 0] = t
        for i in range(1, ORDER):
            s = t * c[:, i] + d[i].astype(np.float32)
            t = t + s
            ts_full[:, i] = t

    in_maps = []
    for cidx in range(NCORES):
        m = dict(shared)
        m["xs"] = np.ascontiguousarray(x_bf[cidx * ROWS : (cidx + 1) * ROWS, :])
        if HOST_DOTS:
            tsc = ts_full[cidx * ROWS : (cidx + 1) * ROWS, :]  # [ROWS, 3]
            # row r = k*P + p -> tsd[p, k*ORDER + i]
            tsd = np.ascontiguousarray(
                tsc.reshape(NT, P, ORDER).transpose(1, 0, 2).reshape(P, NT * ORDER)
            )
            m["tsd"] = tsd
        in_maps.append(m)
    return in_maps


def _run(x, w, b, trace=False):
    from concourse.bass_utils import run_bass_kernel_spmd

    nc = _get_program()
    in_maps = _prep_inputs(x, w, b)
    res = run_bass_kernel_spmd(nc, in_maps, core_ids=list(range(NCORES)), trace=trace)
    outs = [np.asarray(r["out"]) for r in res.results]  # each [ORDER, ROWS, N] bf16
    full = np.concatenate(outs, axis=1)  # [ORDER, B, N]
    return (
        tuple(np.ascontiguousarray(full[i].astype(np.float32)) for i in range(ORDER)),
        res,
    )


def kernel(x, w, b):
    outs, _ = _run(x, w, b, trace=False)
    return outs
